# revision 1
# baseline (speedup 1.0000x reference)
"""Trainium2 Bass kernel for nn_NodeEdgeCrossAttention.

Strategy (dst-sharded, zero-collective):
  - Host sorts edges by destination node, assigns nodes to 8 cores with
    balanced padded-edge counts, and packs each node's edge run (padded to a
    multiple of 32) into 512-column chunks using a slot pattern shared by all
    cores (SPMD requires one program).  Each chunk holds at most 8 slots;
    slot s of chunk c gets global index c*8+s.
  - Scores fold Wq/Wk into per-node M matrices (score = M[dst] . k_raw), so
    no k-projection or q-gather is needed.  bk cancels by softmax shift
    invariance; bv folds through Wo into bo because sum(attn) == 1.
  - Per chunk: one fused kvs DMA (k | v | one-hot S), per-slot score matmuls,
    one exp, one DMA-transpose for edge-major exp values, 4 v-projection
    matmuls, one fused weighted-v multiply, and 4 segment matmuls with the
    one-hot S slot columns as weights accumulating [8 slots, 144] in PSUM
    (seg sums and softmax denominators together).  Park groups of 3 chunks
    drain to a DRAM scratch by DMA.
  - Numerics: fp16 for linear tensors, bf16 for exp-range tensors, fp32
    accumulation; validated at ~2e-3 max relative error.
"""

import numpy as np

N, E, DIM, HEADS = 10000, 640000, 128, 4
DH = DIM // HEADS
NCORES = 8
CHUNK = 512
TILE = 128
SCALE = DH ** -0.5
SP = 16              # exp staging columns per tile
PW = DIM + HEADS     # 132: per-tile rhs width (exv | exE)
GPC = 3              # chunks per PSUM park group


class Plan:
    pass


def _make_plan(dst):
    """Pack nodes into a chunk/slot layout shared across all 8 cores."""
    deg = np.bincount(dst, minlength=N)
    if deg.max() > 128:
        raise NotImplementedError(f"max degree {deg.max()} > 128 needs node splitting")
    Rn = np.maximum(np.ceil(deg / 32.0).astype(np.int64), 1) * 32

    order = np.argsort(-Rn, kind="stable")
    loads = np.zeros(NCORES, np.int64)
    core_nodes = [[] for _ in range(NCORES)]
    for n in order:
        c = int(loads.argmin())
        core_nodes[c].append(int(n))
        loads[c] += Rn[n]

    # Shared slot pattern = elementwise max over cores' (desc-sorted) R seqs.
    L = max(len(cn) for cn in core_nodes)
    pat = np.zeros(L, np.int64)
    for cn in core_nodes:
        r = Rn[np.array(cn, np.int64)]
        pat[: len(r)] = np.maximum(pat[: len(r)], r)

    slots = []           # {R, chunk, col0, pi}
    chunks = []          # {slots: [slot indices]}
    cur = {"slots": []}
    rem = CHUNK
    pi = 0
    while pi < L:
        R = int(pat[pi])
        if R <= rem:
            cur["slots"].append(len(slots))
            slots.append({"R": R, "chunk": len(chunks), "col0": CHUNK - rem, "pi": pi})
            rem -= R
            pi += 1
        else:
            if rem > 0:
                cur["slots"].append(len(slots))
                slots.append({"R": rem, "chunk": len(chunks),
                              "col0": CHUNK - rem, "pi": -1})
            chunks.append(cur)
            cur = {"slots": []}
            rem = CHUNK
    if rem > 0 and rem < CHUNK:
        cur["slots"].append(len(slots))
        slots.append({"R": rem, "chunk": len(chunks), "col0": CHUNK - rem, "pi": -1})
    if cur["slots"]:
        chunks.append(cur)

    max_ns = 0
    for ch in chunks:
        ch["ns"] = len(ch["slots"])
        max_ns = max(max_ns, ch["ns"])

    p = Plan()
    p.sl = max_ns                                    # slot positions per chunk
    p.kvw = 2 * CHUNK + 4 * p.sl
    p.deg = deg
    p.core_nodes = core_nodes
    p.slots = slots
    p.chunks = chunks
    p.nchunks = len(chunks)
    p.cols = p.nchunks * CHUNK
    p.nslot = p.nchunks * p.sl                       # sparse slot space
    p.nslot_b = ((p.nslot + TILE - 1) // TILE) * TILE    # 128-padded
    p.nsp = ((p.nslot + CHUNK - 1) // CHUNK) * CHUNK     # 512-padded
    return p


def _pack_core_inputs(plan, c, k_edges, v_edges, q_nodes, edges_of):
    """Per-core fused kvs [128, nchunks*KVW] f16, qT [128, nsp] f16, qslot."""
    import ml_dtypes
    cols = plan.cols
    edge_order = np.full(cols, -1, np.int64)
    qslot = np.full(plan.nslot, -1, np.int64)
    cn = plan.core_nodes[c]
    for ch_i, ch in enumerate(plan.chunks):
        for j, sidx in enumerate(ch["slots"]):
            s = plan.slots[sidx]
            if s["pi"] < 0 or s["pi"] >= len(cn):
                continue
            node = cn[s["pi"]]
            d = plan.deg[node]
            g0 = ch_i * CHUNK + s["col0"]
            edge_order[g0: g0 + d] = edges_of[node]
            qslot[ch_i * plan.sl + j] = node

    valid = edge_order >= 0
    idx = np.where(valid, edge_order, 0)
    kT = np.where(valid[:, None], k_edges[idx], 0.0).astype(np.float16).T
    vT = np.where(valid[:, None], v_edges[idx], 0.0).astype(np.float16).T

    # one-hot S: [128, nchunks*4*SLOTS], col (chunk, tile, slot_j)
    S = np.zeros((TILE, plan.nchunks * 4 * plan.sl), np.float32)
    for ci, ch in enumerate(plan.chunks):
        for j, sidx in enumerate(ch["slots"]):
            s = plan.slots[sidx]
            if s["pi"] < 0 or s["pi"] >= len(cn):
                continue
            d = int(plan.deg[cn[s["pi"]]])
            for t in range(4):
                lo = max(s["col0"], t * TILE)
                hi = min(s["col0"] + d, (t + 1) * TILE)
                if lo < hi:
                    S[lo - t * TILE:hi - t * TILE, (ci * 4 + t) * plan.sl + j] = 1.0
    Sbits = S.astype(ml_dtypes.bfloat16).view(np.float16)

    kvs = np.empty((TILE, plan.nchunks * plan.kvw), np.float16)
    kc = kT.reshape(TILE, plan.nchunks, CHUNK)
    vc = vT.reshape(TILE, plan.nchunks, CHUNK)
    sc = Sbits.reshape(TILE, plan.nchunks, 4 * plan.sl)
    kvw = kvs.reshape(TILE, plan.nchunks, plan.kvw)
    kvw[:, :, 0:CHUNK] = kc
    kvw[:, :, CHUNK:2 * CHUNK] = vc
    kvw[:, :, 2 * CHUNK:plan.kvw] = sc

    qvalid = qslot >= 0
    qidx = np.where(qvalid, qslot, 0)
    qT = np.zeros((DIM, plan.nsp), np.float16)
    qT[:, : plan.nslot] = np.where(qvalid[:, None], q_nodes[qidx], 0.0
                                   ).astype(np.float16).T
    return kvs, qT, qslot


# ---------------------------------------------------------------------------
# Device kernel emission
# ---------------------------------------------------------------------------

def _build_module(plan):
    import concourse.bacc as bacc
    import concourse.mybir as mybir
    import concourse.tile as tile
    from contextlib import ExitStack

    f16 = mybir.dt.float16
    bf = mybir.dt.bfloat16
    f32 = mybir.dt.float32
    NSP = plan.nsp
    NBLK = plan.nslot_b // TILE
    CW = PW              # 132 scratch row width
    SL = plan.sl
    KVW = plan.kvw

    nc = bacc.Bacc("TRN2", debug=False, num_devices=NCORES)

    kvs_d = nc.dram_tensor("kvs", [TILE, plan.nchunks * KVW], f16,
                           kind="ExternalInput")
    qT_d = nc.dram_tensor("qT", [DIM, NSP], f16, kind="ExternalInput")
    Wq_d = nc.dram_tensor("Wq", [DIM, DIM], f16, kind="ExternalInput")
    WkTs_d = nc.dram_tensor("WkTs", [DIM, DIM], f16, kind="ExternalInput")
    Wv_d = nc.dram_tensor("Wv", [DIM, DIM], f16, kind="ExternalInput")
    Wo_d = nc.dram_tensor("Wo", [DIM, DIM], f32, kind="ExternalInput")
    Hm_d = nc.dram_tensor("Hm", [DIM, HEADS], f16, kind="ExternalInput")
    ID_d = nc.dram_tensor("ID", [DIM, DIM], f32, kind="ExternalInput")
    I4_d = nc.dram_tensor("I4", [HEADS, HEADS], bf, kind="ExternalInput")
    bq_d = nc.dram_tensor("bq", [DIM, 1], f32, kind="ExternalInput")
    bo_d = nc.dram_tensor("bo", [DIM, 1], f32, kind="ExternalInput")
    accD = nc.dram_tensor("accD", [plan.nslot, CW], f32, kind="Internal")
    outT_d = nc.dram_tensor("outT", [DIM, NSP], f32, kind="ExternalOutput")

    Exp = mybir.ActivationFunctionType.Exp
    Ident = mybir.ActivationFunctionType.Identity
    mult = mybir.AluOpType.mult
    amax = mybir.AluOpType.max

    with ExitStack() as ctx:
        tc = ctx.enter_context(tile.TileContext(nc))
        cp = ctx.enter_context(tc.tile_pool(name="const", bufs=1))
        sp = ctx.enter_context(tc.tile_pool(name="persist", bufs=1))
        iop = ctx.enter_context(tc.tile_pool(name="io", bufs=4))
        xp = ctx.enter_context(tc.tile_pool(name="work", bufs=4))
        pp = ctx.enter_context(tc.tile_pool(name="ps", bufs=2, space="PSUM"))

        def dmac(tile_ap, dram_ap):
            nc.sync.dma_start(out=tile_ap, in_=dram_ap)

        Wq_sb = cp.tile([DIM, DIM], f16); dmac(Wq_sb[:], Wq_d[:, :])
        WkTs_sb = cp.tile([DIM, DIM], f16); dmac(WkTs_sb[:], WkTs_d[:, :])
        Wv_sb = cp.tile([DIM, DIM], f16); dmac(Wv_sb[:], Wv_d[:, :])
        Wo_sb = cp.tile([DIM, DIM], f32); dmac(Wo_sb[:], Wo_d[:, :])
        Hm_sb = cp.tile([DIM, HEADS], f16); dmac(Hm_sb[:], Hm_d[:, :])
        ID_sb = cp.tile([DIM, DIM], f32); dmac(ID_sb[:], ID_d[:, :])
        I4_sb = cp.tile([HEADS, HEADS], bf); dmac(I4_sb[:], I4_d[:, :])
        bq_sb = cp.tile([DIM, 1], f32); dmac(bq_sb[:], bq_d[:, :])
        bo_sb = cp.tile([DIM, 1], f32); dmac(bo_sb[:], bo_d[:, :])
        qT_sb = sp.tile([DIM, NSP], f16); dmac(qT_sb[:], qT_d[:, :])

        qp_sb = sp.tile([DIM, NSP], f16)
        M_sb = sp.tile([DIM, 4 * NSP], f16)

        # ---- Stage A: q projection + bias ----
        for b in range(NSP // CHUNK):
            sl = slice(b * CHUNK, (b + 1) * CHUNK)
            qp_ps = pp.tile([DIM, CHUNK], f32, tag="aux")
            nc.tensor.matmul(out=qp_ps[:], lhsT=Wq_sb[:], rhs=qT_sb[:, sl],
                             start=True, stop=True)
            nc.scalar.activation(out=qp_sb[:, sl], in_=qp_ps[:],
                                 func=Ident, bias=bq_sb[:, 0:1])

        # ---- Stage A: M matrices, 32 slots per group ----
        for g in range(NSP // 32):
            qsl = slice(g * 32, (g + 1) * 32)
            qpm = xp.tile([DIM, TILE], f16, tag="qpm")
            nc.vector.tensor_tensor(
                out=qpm[:].rearrange("p (w h) -> p w h", h=HEADS),
                in0=qp_sb[:, qsl].unsqueeze(-1).to_broadcast([DIM, 32, HEADS]),
                in1=Hm_sb[:, :].unsqueeze(1).to_broadcast([DIM, 32, HEADS]),
                op=mult)
            M_ps = pp.tile([DIM, TILE], f32, tag="aux")
            nc.tensor.matmul(out=M_ps[:], lhsT=WkTs_sb[:], rhs=qpm[:],
                             start=True, stop=True)
            nc.scalar.copy(out=M_sb[:, g * TILE:(g + 1) * TILE], in_=M_ps[:])

        # ---- Steady state ----
        park = None
        for ci, ch in enumerate(plan.chunks):
            kvt = iop.tile([TILE, KVW], f16, tag="kv")
            dmac(kvt[:], kvs_d[:, ci * KVW:(ci + 1) * KVW])
            kc = kvt[:, 0:CHUNK]
            vc = kvt[:, CHUNK:2 * CHUNK]
            Sc = kvt[:, 2 * CHUNK:KVW].bitcast(bf)

            score_ps = pp.tile([HEADS, CHUNK], f32, tag="score")
            for j, sidx in enumerate(ch["slots"]):
                s = plan.slots[sidx]
                g = ci * SL + j
                c0 = s["col0"]
                nc.tensor.matmul(
                    out=score_ps[0:HEADS, c0:c0 + s["R"]],
                    lhsT=M_sb[:, 4 * g:4 * g + 4],
                    rhs=kc[:, c0:c0 + s["R"]],
                    start=True, stop=True)

            ex_sb = xp.tile([HEADS, CHUNK], bf, tag="ex")
            nc.scalar.activation(out=ex_sb[:], in_=score_ps[:], func=Exp)
            exT_ps = pp.tile([TILE, 4 * HEADS], f32, tag="aux")
            for t in range(4):
                nc.tensor.matmul(
                    out=exT_ps[:, 4 * t:4 * t + 4],
                    lhsT=ex_sb[0:HEADS, t * TILE:(t + 1) * TILE],
                    rhs=I4_sb[:], start=True, stop=True)
            exE_sb = xp.tile([TILE, 4 * HEADS], bf, tag="exE")
            nc.scalar.copy(out=exE_sb[:], in_=exT_ps[:])

            vp_ps = pp.tile([TILE, CHUNK], f32, tag="vp")
            for t in range(4):
                nc.tensor.matmul(
                    out=vp_ps[:, t * TILE:(t + 1) * TILE],
                    lhsT=vc[:, t * TILE:(t + 1) * TILE],
                    rhs=Wv_sb[:], start=True, stop=True)

            exF_sb = xp.tile([TILE, 4 * PW], bf, tag="exF")
            exF_t = exF_sb[:].rearrange("p (t c) -> p t c", t=4)
            nc.vector.tensor_tensor(
                out=exF_t[:, :, 0:DIM].rearrange("p t (h d) -> p t h d", h=HEADS),
                in0=vp_ps[:].rearrange("p (t h d) -> p t h d", t=4, h=HEADS),
                in1=exE_sb[:].rearrange("p (t h) -> p t h", t=4)
                    .unsqueeze(-1).to_broadcast([TILE, 4, HEADS, DH]),
                op=mult)
            nc.scalar.copy(
                out=exF_t[:, :, DIM:PW],
                in_=exE_sb[:].rearrange("p (t h) -> p t h", t=4))

            gi = ci % GPC
            if gi == 0:
                park = pp.tile([SL, GPC * CW], f32, tag="park")
            for t in range(4):
                nc.tensor.matmul(
                    out=park[:, gi * CW:(gi + 1) * CW],
                    lhsT=Sc[:, t * SL:(t + 1) * SL],
                    rhs=exF_t[:, t, :],
                    start=(t == 0), stop=(t == 3))
            if gi == GPC - 1 or ci == plan.nchunks - 1:
                g0 = (ci // GPC) * GPC
                used = ci - g0 + 1
                stage = xp.tile([SL, GPC * CW], f32, tag="stage")
                nc.vector.tensor_copy(out=stage[:, 0:used * CW],
                                      in_=park[:, 0:used * CW])
                nc.scalar.dma_start(
                    out=accD[g0 * SL:(ci + 1) * SL, :]
                        .rearrange("(c j) w -> j c w", j=SL),
                    in_=stage[:, 0:used * CW]
                        .rearrange("j (c w) -> j c w", w=CW))

        # ---- Final: read scratch back aligned, normalize, project ----
        accR = sp.tile([TILE, NBLK * CW], f32)
        nc.gpsimd.memset(accR[:], 0.0)
        full = plan.nslot // TILE          # whole 128-row blocks
        if full:
            dmac(accR[:, 0:full * CW]
                 .rearrange("p (b w) -> p b w", w=CW),
                 accD[0:full * TILE, :].rearrange("(b p) w -> p b w", p=TILE))
        tail = plan.nslot - full * TILE
        if tail:
            dmac(accR[0:tail, full * CW:(full + 1) * CW],
                 accD[full * TILE:plan.nslot, :])

        rden_sb = sp.tile([TILE, NBLK * HEADS], f32)
        nc.vector.tensor_scalar(
            out=rden_sb[:].rearrange("p (b h) -> p b h", h=HEADS),
            in0=accR[:].rearrange("p (b w) -> p b w", w=CW)[:, :, DIM:DIM + HEADS],
            scalar1=1e-30, scalar2=None, op0=amax)
        nc.vector.reciprocal(out=rden_sb[:], in_=rden_sb[:])
        agg_sb = sp.tile([TILE, NBLK * DIM], f32)
        nc.vector.tensor_tensor(
            out=agg_sb[:].rearrange("p (b h d) -> p b h d", b=NBLK, h=HEADS),
            in0=accR[:].rearrange("p (b w) -> p b w", w=CW)[:, :, 0:DIM]
                .rearrange("p b (h d) -> p b h d", h=HEADS),
            in1=rden_sb[:].rearrange("p (b h) -> p b h", h=HEADS)
                .unsqueeze(-1).to_broadcast([TILE, NBLK, HEADS, DH]),
            op=mult)
        aggT_sb = sp.tile([TILE, NSP], f32)
        nc.gpsimd.memset(aggT_sb[:], 0.0)
        for b in range(NBLK):
            tp_ps = pp.tile([DIM, TILE], f32, tag="aux")
            nc.tensor.transpose(out=tp_ps[:],
                                in_=agg_sb[:, b * TILE:(b + 1) * TILE],
                                identity=ID_sb[:])
            nc.scalar.copy(out=aggT_sb[:, b * TILE:(b + 1) * TILE], in_=tp_ps[:])
        for b in range(NSP // CHUNK):
            sl = slice(b * CHUNK, (b + 1) * CHUNK)
            out_ps = pp.tile([DIM, CHUNK], f32, tag="aux")
            nc.tensor.matmul(out=out_ps[:], lhsT=Wo_sb[:],
                             rhs=aggT_sb[:, sl], start=True, stop=True)
            osb = xp.tile([DIM, CHUNK], f32, tag="osb")
            nc.scalar.activation(out=osb[:], in_=out_ps[:],
                                 func=Ident, bias=bo_sb[:, 0:1])
            dmac(outT_d[:, sl], osb[:])

    nc.compile()
    return nc


# ---------------------------------------------------------------------------
# Entry point
# ---------------------------------------------------------------------------

def _prepare(inputs):
    q_nodes = np.asarray(inputs["q_nodes"], np.float32)
    k_edges = np.asarray(inputs["k_edges"], np.float32)
    v_edges = np.asarray(inputs["v_edges"], np.float32)
    Wq = np.asarray(inputs["Wq"], np.float32)
    bq = np.asarray(inputs["bq"], np.float32)
    Wk = np.asarray(inputs["Wk"], np.float32)
    Wv = np.asarray(inputs["Wv"], np.float32)
    bv = np.asarray(inputs["bv"], np.float32)
    Wo = np.asarray(inputs["Wo"], np.float32)
    bo = np.asarray(inputs["bo"], np.float32)
    dst = np.asarray(inputs["edge_index"])[0].astype(np.int64)

    plan = _make_plan(dst)

    eorder = np.argsort(dst, kind="stable")
    starts = np.zeros(N + 1, np.int64)
    np.cumsum(np.bincount(dst, minlength=N), out=starts[1:])
    edges_of = [eorder[starts[n]: starts[n + 1]] for n in range(N)]

    consts = {
        "Wq": Wq.astype(np.float16),
        "WkTs": np.ascontiguousarray((Wk * SCALE).T).astype(np.float16),
        "Wv": Wv.astype(np.float16),
        "Wo": np.ascontiguousarray(Wo),
        "Hm": (np.arange(DIM)[:, None] // DH == np.arange(HEADS)[None, :]
               ).astype(np.float16),
        "ID": np.eye(DIM, dtype=np.float32),
        "I4": np.eye(HEADS).astype(__import__('ml_dtypes').bfloat16),
        "bq": bq.reshape(DIM, 1).astype(np.float32),
        # sum(attn)==1 folds bv through Wo: out = (segv/den)@Wo + (bv@Wo + bo)
        "bo": (bv @ Wo + bo).reshape(DIM, 1).astype(np.float32),
    }
    return plan, dst, edges_of, consts, q_nodes, k_edges, v_edges, bo


def kernel(**inputs):
    from concourse.bass_utils import run_bass_kernel_spmd

    (plan, dst, edges_of, consts, q_nodes, k_edges, v_edges, bo) = _prepare(inputs)

    nc = _build_module(plan)

    in_maps = []
    slot_maps = []
    for c in range(NCORES):
        kvs, qT, qslot = _pack_core_inputs(plan, c, k_edges, v_edges,
                                           q_nodes, edges_of)
        m = {"kvs": kvs, "qT": qT}
        m.update(consts)
        in_maps.append(m)
        slot_maps.append(qslot)

    res = run_bass_kernel_spmd(nc, in_maps, core_ids=list(range(NCORES)))
    global LAST_RESULTS
    LAST_RESULTS = res

    out = np.zeros((N, DIM), np.float32)
    for c in range(NCORES):
        outT = res.results[c]["outT"]          # [DIM, nsp]
        qslot = slot_maps[c]
        valid = qslot >= 0
        out[qslot[valid]] = outT[:, : plan.nslot].T[valid]
    deg0 = plan.deg == 0
    if deg0.any():
        out[deg0] = bo
    return out



# revision 8
# speedup vs baseline: 1.1114x; 1.1114x over previous
"""Trainium2 Bass kernel for nn_NodeEdgeCrossAttention.

Strategy (dst-sharded, zero-collective):
  - Host sorts edges by destination node, assigns nodes to 8 cores with
    balanced padded-edge counts, and bin-packs each node's edge run (padded
    to a multiple of 32, max 128) into 128-column tiles, 4 tiles per
    512-column chunk, using a slot pattern shared by all cores (SPMD
    requires one program).  No slot spans a tile boundary.
  - Scores fold Wq/Wk into per-node M matrices (score = M[dst] . k_raw), so
    no k-projection or q-gather is needed.  bk cancels by softmax shift
    invariance; bv folds through Wo into bo because sum(attn) == 1.
  - Scores are computed EDGE-MAJOR: per slot, the kc slice is the matmul
    stationary operand and the node's 4 M columns are moving, so exp sees a
    [128, 16] (tile x head) PSUM tile directly -- no transpose matmuls and a
    tiny activation instead of a [4, 512] one.
  - Per chunk: one fused kvs DMA (k | v | one-hot S), per-slot score
    matmuls, one exp, 4 v-projection matmuls, one fused weighted-v multiply,
    and 4 segment matmuls with the one-hot S slot columns as stationary
    accumulating [SL slots, 132] in PSUM (seg sums and softmax denominators
    together).  Segment matmuls for chunk ci are emitted one iteration
    late so the PE never stalls on the scalar->vector chain.  Park groups
    of 3 chunks drain to a DRAM scratch by DMA.
  - Numerics: fp16 for linear tensors, bf16 for exp-range tensors, fp32
    accumulation; validated at ~2e-3 max relative error.
"""

import numpy as np

N, E, DIM, HEADS = 10000, 640000, 128, 4
DH = DIM // HEADS
NCORES = 8
CHUNK = 512
TILE = 128
SCALE = DH ** -0.5
PW = DIM + HEADS     # 132: per-tile rhs width (exv | exE)
GPC = 3              # chunks per PSUM park group


class Plan:
    pass


def _make_plan(dst):
    """Pack nodes into a chunk/slot layout shared across all 8 cores."""
    deg = np.bincount(dst, minlength=N)
    if deg.max() > 128:
        raise NotImplementedError(f"max degree {deg.max()} > 128 needs node splitting")
    Rn = np.maximum(np.ceil(deg / 32.0).astype(np.int64), 1) * 32

    order = np.argsort(-Rn, kind="stable")
    loads = np.zeros(NCORES, np.int64)
    core_nodes = [[] for _ in range(NCORES)]
    for n in order:
        c = int(loads.argmin())
        core_nodes[c].append(int(n))
        loads[c] += Rn[n]

    # Shared slot pattern = elementwise max over cores' (desc-sorted) R seqs.
    L = max(len(cn) for cn in core_nodes)
    pat = np.zeros(L, np.int64)
    for cn in core_nodes:
        r = Rn[np.array(cn, np.int64)]
        pat[: len(r)] = np.maximum(pat[: len(r)], r)

    slots = []           # {R, chunk, col0, pi}
    chunks = []          # {slots: [slot indices]}
    cur = {"slots": []}
    rem = CHUNK
    pi = 0
    while pi < L:
        R = int(pat[pi])
        if R <= rem:
            cur["slots"].append(len(slots))
            slots.append({"R": R, "chunk": len(chunks), "col0": CHUNK - rem, "pi": pi})
            rem -= R
            pi += 1
        else:
            if rem > 0:
                cur["slots"].append(len(slots))
                slots.append({"R": rem, "chunk": len(chunks),
                              "col0": CHUNK - rem, "pi": -1})
            chunks.append(cur)
            cur = {"slots": []}
            rem = CHUNK
    if rem > 0 and rem < CHUNK:
        cur["slots"].append(len(slots))
        slots.append({"R": rem, "chunk": len(chunks), "col0": CHUNK - rem, "pi": -1})
    if cur["slots"]:
        chunks.append(cur)

    nchunks = len(chunks)
    max_ns = 0
    for ch in chunks:
        ch["ns"] = len(ch["slots"])
        max_ns = max(max_ns, ch["ns"])

    p = Plan()
    p.sl = max_ns                                    # slot positions per chunk
    p.kvw = 2 * CHUNK + 4 * p.sl
    p.deg = deg
    p.core_nodes = core_nodes
    p.slots = slots
    p.chunks = chunks
    p.nchunks = nchunks
    p.cols = nchunks * CHUNK
    p.nslot = nchunks * p.sl                         # sparse slot space
    p.nslot_b = ((p.nslot + TILE - 1) // TILE) * TILE    # 128-padded
    p.nsp = ((p.nslot + CHUNK - 1) // CHUNK) * CHUNK     # 512-padded
    return p


def _pack_core_inputs(plan, c, k_edges, v_edges, q_nodes, edges_of):
    """Per-core fused kvs [128, nchunks*KVW] f16, qT [128, nsp] f16, qslot."""
    import ml_dtypes
    cols = plan.cols
    edge_order = np.full(cols, -1, np.int64)
    qslot = np.full(plan.nslot, -1, np.int64)
    cn = plan.core_nodes[c]
    for ci, ch in enumerate(plan.chunks):
        for j, sidx in enumerate(ch["slots"]):
            s = plan.slots[sidx]
            if s["pi"] < 0 or s["pi"] >= len(cn):
                continue
            node = cn[s["pi"]]
            d = plan.deg[node]
            g0 = ci * CHUNK + s["col0"]
            edge_order[g0: g0 + d] = edges_of[node]
            qslot[ci * plan.sl + j] = node

    valid = edge_order >= 0
    idx = np.where(valid, edge_order, 0)
    kT = np.where(valid[:, None], k_edges[idx], 0.0).astype(np.float16).T
    vT = np.where(valid[:, None], v_edges[idx], 0.0).astype(np.float16).T

    # one-hot S: [128, nchunks*4*SL], col (chunk, tile, slot_j)
    S = np.zeros((TILE, plan.nchunks * 4 * plan.sl), np.float32)
    for ci, ch in enumerate(plan.chunks):
        for j, sidx in enumerate(ch["slots"]):
            s = plan.slots[sidx]
            if s["pi"] < 0 or s["pi"] >= len(cn):
                continue
            d = int(plan.deg[cn[s["pi"]]])
            for t in range(4):
                lo = max(s["col0"], t * TILE)
                hi = min(s["col0"] + d, (t + 1) * TILE)
                if lo < hi:
                    S[lo - t * TILE:hi - t * TILE, (ci * 4 + t) * plan.sl + j] = 1.0
    Sbits = S.astype(ml_dtypes.bfloat16).view(np.float16)

    kvs = np.empty((TILE, plan.nchunks * plan.kvw), np.float16)
    kc = kT.reshape(TILE, plan.nchunks, CHUNK)
    vc = vT.reshape(TILE, plan.nchunks, CHUNK)
    sc = Sbits.reshape(TILE, plan.nchunks, 4 * plan.sl)
    kvw = kvs.reshape(TILE, plan.nchunks, plan.kvw)
    kvw[:, :, 0:CHUNK] = kc
    kvw[:, :, CHUNK:2 * CHUNK] = vc
    kvw[:, :, 2 * CHUNK:plan.kvw] = sc

    qvalid = qslot >= 0
    qidx = np.where(qvalid, qslot, 0)
    qT = np.zeros((DIM, plan.nsp), np.float16)
    qT[:, : plan.nslot] = np.where(qvalid[:, None], q_nodes[qidx], 0.0
                                   ).astype(np.float16).T
    return kvs, qT, qslot


# ---------------------------------------------------------------------------
# Device kernel emission
# ---------------------------------------------------------------------------

def _build_module(plan):
    import concourse.bacc as bacc
    import concourse.mybir as mybir
    import concourse.tile as tile
    from contextlib import ExitStack

    f16 = mybir.dt.float16
    bf = mybir.dt.bfloat16
    f32 = mybir.dt.float32
    NSP = plan.nsp
    NBLK = plan.nslot_b // TILE
    CW = PW              # 132 scratch row width
    SL = plan.sl
    KVW = plan.kvw

    nc = bacc.Bacc("TRN2", debug=False, num_devices=NCORES)

    kvs_d = nc.dram_tensor("kvs", [TILE, plan.nchunks * KVW], f16,
                           kind="ExternalInput")
    qT_d = nc.dram_tensor("qT", [DIM, NSP], f16, kind="ExternalInput")
    Wq_d = nc.dram_tensor("Wq", [DIM, DIM], f16, kind="ExternalInput")
    WkTs_d = nc.dram_tensor("WkTs", [DIM, DIM], f16, kind="ExternalInput")
    Wv_d = nc.dram_tensor("Wv", [DIM, DIM], f16, kind="ExternalInput")
    Wo_d = nc.dram_tensor("Wo", [DIM, DIM], f32, kind="ExternalInput")
    Hm_d = nc.dram_tensor("Hm", [DIM, HEADS], f16, kind="ExternalInput")
    ID_d = nc.dram_tensor("ID", [DIM, DIM], f32, kind="ExternalInput")
    bq_d = nc.dram_tensor("bq", [DIM, 1], f32, kind="ExternalInput")
    bo_d = nc.dram_tensor("bo", [DIM, 1], f32, kind="ExternalInput")
    accD = nc.dram_tensor("accD", [plan.nslot, CW], f32, kind="Internal")
    outT_d = nc.dram_tensor("outT", [DIM, NSP], f32, kind="ExternalOutput")

    Exp = mybir.ActivationFunctionType.Exp
    Ident = mybir.ActivationFunctionType.Identity
    mult = mybir.AluOpType.mult
    amax = mybir.AluOpType.max

    with ExitStack() as ctx:
        tc = ctx.enter_context(tile.TileContext(nc))
        cp = ctx.enter_context(tc.tile_pool(name="const", bufs=1))
        sp = ctx.enter_context(tc.tile_pool(name="persist", bufs=1))
        iop = ctx.enter_context(tc.tile_pool(name="io", bufs=4))
        xp = ctx.enter_context(tc.tile_pool(name="work", bufs=4))
        pp = ctx.enter_context(tc.tile_pool(name="ps", bufs=2, space="PSUM"))

        def dmac(tile_ap, dram_ap):
            nc.sync.dma_start(out=tile_ap, in_=dram_ap)

        Wq_sb = cp.tile([DIM, DIM], f16); dmac(Wq_sb[:], Wq_d[:, :])
        WkTs_sb = cp.tile([DIM, DIM], f16); dmac(WkTs_sb[:], WkTs_d[:, :])
        Wv_sb = cp.tile([DIM, DIM], f16); dmac(Wv_sb[:], Wv_d[:, :])
        Wo_sb = cp.tile([DIM, DIM], f32); dmac(Wo_sb[:], Wo_d[:, :])
        Hm_sb = cp.tile([DIM, HEADS], f16); dmac(Hm_sb[:], Hm_d[:, :])
        ID_sb = cp.tile([DIM, DIM], f32); dmac(ID_sb[:], ID_d[:, :])
        bq_sb = cp.tile([DIM, 1], f32); dmac(bq_sb[:], bq_d[:, :])
        bo_sb = cp.tile([DIM, 1], f32); dmac(bo_sb[:], bo_d[:, :])
        qT_sb = sp.tile([DIM, NSP], f16); dmac(qT_sb[:], qT_d[:, :])

        qp_sb = sp.tile([DIM, NSP], f16)
        M_sb = sp.tile([DIM, 4 * NSP], f16)

        # ---- Stage A: q projection + bias ----
        for b in range(NSP // CHUNK):
            sl = slice(b * CHUNK, (b + 1) * CHUNK)
            qp_ps = pp.tile([DIM, CHUNK], f32, tag="aux")
            nc.tensor.matmul(out=qp_ps[:], lhsT=Wq_sb[:], rhs=qT_sb[:, sl],
                             start=True, stop=True)
            nc.scalar.activation(out=qp_sb[:, sl], in_=qp_ps[:],
                                 func=Ident, bias=bq_sb[:, 0:1])

        # ---- Stage A: M matrices, 32 slots per group ----
        for g in range(NSP // 32):
            qsl = slice(g * 32, (g + 1) * 32)
            qpm = xp.tile([DIM, TILE], f16, tag="qpm")
            nc.vector.tensor_tensor(
                out=qpm[:].rearrange("p (w h) -> p w h", h=HEADS),
                in0=qp_sb[:, qsl].unsqueeze(-1).to_broadcast([DIM, 32, HEADS]),
                in1=Hm_sb[:, :].unsqueeze(1).to_broadcast([DIM, 32, HEADS]),
                op=mult)
            M_ps = pp.tile([DIM, TILE], f32, tag="aux")
            nc.tensor.matmul(out=M_ps[:], lhsT=WkTs_sb[:], rhs=qpm[:],
                             start=True, stop=True)
            nc.scalar.copy(out=M_sb[:, g * TILE:(g + 1) * TILE], in_=M_ps[:])

        # ---- Steady state (seg matmuls lag one chunk behind) ----
        park = None
        pend = {}
        for ci in range(plan.nchunks + 1):
            if ci < plan.nchunks:
                ch = plan.chunks[ci]
                kvt = iop.tile([TILE, KVW], f16, tag="kv")
                dmac(kvt[:], kvs_d[:, ci * KVW:(ci + 1) * KVW])
                kc = kvt[:, 0:CHUNK]
                vc = kvt[:, CHUNK:2 * CHUNK]

                ex_ps = pp.tile([TILE, 4 * HEADS], f32, tag="score")
                pieces = []                         # (order, lo, hi, g)
                for j, sidx in enumerate(ch["slots"]):
                    s = plan.slots[sidx]
                    g = ci * SL + j
                    lo = s["col0"]
                    end = s["col0"] + s["R"]
                    while lo < end:                 # split at 128-col tiles
                        t = lo // TILE
                        hi = min(end, (t + 1) * TILE)
                        r0 = lo - t * TILE
                        # PSUM out rows must start at partition 0/32/64 and
                        # not cross the 64 row from 32.  Widen a 96-start
                        # piece to 64 (its 64:96 garbage rows are
                        # overwritten by the true owner, emitted later);
                        # split a 32-start piece at row 64.
                        if r0 == 96:
                            pieces.append((0, lo - 32, hi, g))
                        elif r0 == 32 and hi - lo > 32:
                            pieces.append((1, lo, lo + 32, g))
                            pieces.append((1, lo + 32, hi, g))
                        else:
                            pieces.append((1, lo, hi, g))
                        lo = hi
                for _, lo, hi, g in sorted(pieces, key=lambda x: x[0]):
                    t = lo // TILE
                    nc.tensor.matmul(
                        out=ex_ps[lo - t * TILE:hi - t * TILE,
                                  4 * t:4 * t + 4],
                        lhsT=kc[:, lo:hi],
                        rhs=M_sb[:, 4 * g:4 * g + 4],
                        start=True, stop=True)

                exE_sb = xp.tile([TILE, 4 * HEADS], bf, tag="exE")
                nc.scalar.activation(out=exE_sb[:], in_=ex_ps[:], func=Exp)

                vp_ps = pp.tile([TILE, CHUNK], f32, tag="vp")
                for t in range(4):
                    nc.tensor.matmul(
                        out=vp_ps[:, t * TILE:(t + 1) * TILE],
                        lhsT=vc[:, t * TILE:(t + 1) * TILE],
                        rhs=Wv_sb[:], start=True, stop=True)

                exF_sb = xp.tile([TILE, 4 * PW], bf, tag="exF")
                exF_t = exF_sb[:].rearrange("p (t c) -> p t c", t=4)
                nc.vector.tensor_tensor(
                    out=exF_t[:, :, 0:DIM].rearrange("p t (h d) -> p t h d", h=HEADS),
                    in0=vp_ps[:].rearrange("p (t h d) -> p t h d", t=4, h=HEADS),
                    in1=exE_sb[:].rearrange("p (t h) -> p t h", t=4)
                        .unsqueeze(-1).to_broadcast([TILE, 4, HEADS, DH]),
                    op=mult)
                nc.scalar.copy(
                    out=exF_t[:, :, DIM:PW],
                    in_=exE_sb[:].rearrange("p (t h) -> p t h", t=4))
                pend[ci] = (kvt, exF_t)

            if ci >= 1:
                cj = ci - 1
                kvt_j, exF_j = pend.pop(cj)
                Sc = kvt_j[:, 2 * CHUNK:KVW].bitcast(bf)
                gi = cj % GPC
                if gi == 0:
                    park = pp.tile([SL, GPC * CW], f32, tag="park")
                for t in range(4):
                    nc.tensor.matmul(
                        out=park[:, gi * CW:(gi + 1) * CW],
                        lhsT=Sc[:, t * SL:(t + 1) * SL],
                        rhs=exF_j[:, t, :],
                        start=(t == 0), stop=(t == 3))
                if gi == GPC - 1 or cj == plan.nchunks - 1:
                    g0 = (cj // GPC) * GPC
                    used = cj - g0 + 1
                    stage = xp.tile([SL, GPC * CW], f32, tag="stage")
                    nc.vector.tensor_copy(out=stage[:, 0:used * CW],
                                          in_=park[:, 0:used * CW])
                    nc.scalar.dma_start(
                        out=accD[g0 * SL:(cj + 1) * SL, :]
                            .rearrange("(c j) w -> j c w", j=SL),
                        in_=stage[:, 0:used * CW]
                            .rearrange("j (c w) -> j c w", w=CW))

        # ---- Final: read scratch back aligned, normalize, project ----
        accR = sp.tile([TILE, NBLK * CW], f32)
        nc.gpsimd.memset(accR[:], 0.0)
        full = plan.nslot // TILE          # whole 128-row blocks
        if full:
            dmac(accR[:, 0:full * CW]
                 .rearrange("p (b w) -> p b w", w=CW),
                 accD[0:full * TILE, :].rearrange("(b p) w -> p b w", p=TILE))
        tail = plan.nslot - full * TILE
        if tail:
            dmac(accR[0:tail, full * CW:(full + 1) * CW],
                 accD[full * TILE:plan.nslot, :])

        rden_sb = sp.tile([TILE, NBLK * HEADS], f32)
        nc.vector.tensor_scalar(
            out=rden_sb[:].rearrange("p (b h) -> p b h", h=HEADS),
            in0=accR[:].rearrange("p (b w) -> p b w", w=CW)[:, :, DIM:DIM + HEADS],
            scalar1=1e-30, scalar2=None, op0=amax)
        nc.vector.reciprocal(out=rden_sb[:], in_=rden_sb[:])
        agg_sb = sp.tile([TILE, NBLK * DIM], f32)
        nc.vector.tensor_tensor(
            out=agg_sb[:].rearrange("p (b h d) -> p b h d", b=NBLK, h=HEADS),
            in0=accR[:].rearrange("p (b w) -> p b w", w=CW)[:, :, 0:DIM]
                .rearrange("p b (h d) -> p b h d", h=HEADS),
            in1=rden_sb[:].rearrange("p (b h) -> p b h", h=HEADS)
                .unsqueeze(-1).to_broadcast([TILE, NBLK, HEADS, DH]),
            op=mult)
        aggT_sb = sp.tile([TILE, NSP], f32)
        nc.gpsimd.memset(aggT_sb[:], 0.0)
        for b in range(NBLK):
            tp_ps = pp.tile([DIM, TILE], f32, tag="aux")
            nc.tensor.transpose(out=tp_ps[:],
                                in_=agg_sb[:, b * TILE:(b + 1) * TILE],
                                identity=ID_sb[:])
            nc.scalar.copy(out=aggT_sb[:, b * TILE:(b + 1) * TILE], in_=tp_ps[:])
        for b in range(NSP // CHUNK):
            sl = slice(b * CHUNK, (b + 1) * CHUNK)
            out_ps = pp.tile([DIM, CHUNK], f32, tag="aux")
            nc.tensor.matmul(out=out_ps[:], lhsT=Wo_sb[:],
                             rhs=aggT_sb[:, sl], start=True, stop=True)
            osb = xp.tile([DIM, CHUNK], f32, tag="osb")
            nc.scalar.activation(out=osb[:], in_=out_ps[:],
                                 func=Ident, bias=bo_sb[:, 0:1])
            dmac(outT_d[:, sl], osb[:])

    nc.compile()
    return nc


# ---------------------------------------------------------------------------
# Entry point
# ---------------------------------------------------------------------------

def _prepare(inputs):
    q_nodes = np.asarray(inputs["q_nodes"], np.float32)
    k_edges = np.asarray(inputs["k_edges"], np.float32)
    v_edges = np.asarray(inputs["v_edges"], np.float32)
    Wq = np.asarray(inputs["Wq"], np.float32)
    bq = np.asarray(inputs["bq"], np.float32)
    Wk = np.asarray(inputs["Wk"], np.float32)
    Wv = np.asarray(inputs["Wv"], np.float32)
    bv = np.asarray(inputs["bv"], np.float32)
    Wo = np.asarray(inputs["Wo"], np.float32)
    bo = np.asarray(inputs["bo"], np.float32)
    dst = np.asarray(inputs["edge_index"])[0].astype(np.int64)

    plan = _make_plan(dst)

    eorder = np.argsort(dst, kind="stable")
    starts = np.zeros(N + 1, np.int64)
    np.cumsum(np.bincount(dst, minlength=N), out=starts[1:])
    edges_of = [eorder[starts[n]: starts[n + 1]] for n in range(N)]

    consts = {
        "Wq": Wq.astype(np.float16),
        "WkTs": np.ascontiguousarray((Wk * SCALE).T).astype(np.float16),
        "Wv": Wv.astype(np.float16),
        "Wo": np.ascontiguousarray(Wo),
        "Hm": (np.arange(DIM)[:, None] // DH == np.arange(HEADS)[None, :]
               ).astype(np.float16),
        "ID": np.eye(DIM, dtype=np.float32),
        "bq": bq.reshape(DIM, 1).astype(np.float32),
        # sum(attn)==1 folds bv through Wo: out = (segv/den)@Wo + (bv@Wo + bo)
        "bo": (bv @ Wo + bo).reshape(DIM, 1).astype(np.float32),
    }
    return plan, dst, edges_of, consts, q_nodes, k_edges, v_edges, bo


def kernel(**inputs):
    from concourse.bass_utils import run_bass_kernel_spmd

    (plan, dst, edges_of, consts, q_nodes, k_edges, v_edges, bo) = _prepare(inputs)

    nc = _build_module(plan)

    in_maps = []
    slot_maps = []
    for c in range(NCORES):
        kvs, qT, qslot = _pack_core_inputs(plan, c, k_edges, v_edges,
                                           q_nodes, edges_of)
        m = {"kvs": kvs, "qT": qT}
        m.update(consts)
        in_maps.append(m)
        slot_maps.append(qslot)

    res = run_bass_kernel_spmd(nc, in_maps, core_ids=list(range(NCORES)))
    global LAST_RESULTS
    LAST_RESULTS = res

    out = np.zeros((N, DIM), np.float32)
    for c in range(NCORES):
        outT = res.results[c]["outT"]          # [DIM, nsp]
        qslot = slot_maps[c]
        valid = qslot >= 0
        out[qslot[valid]] = outT[:, : plan.nslot].T[valid]
    deg0 = plan.deg == 0
    if deg0.any():
        out[deg0] = bo
    return out
